# revision 14
# baseline (speedup 1.0000x reference)
"""AttentionConv2d Trainium2 kernel — 8-core batch-data-parallel (v4).

Per-core (one image each):
  - x uploaded host-padded bf16 [2, 128, 34, 34]; qkv/vT/conv3x3 read
    strided views (no device casts / pad build)
  - qkv 1x1 conv (q, k) + transposed V projection as matmuls
  - rel-position G/W values via 4-way diagonal tile_position matmuls with
    compact per-head kr window tables (256KB vs 2MB expanded)
  - q / G / W staged directly into persistent per-head rhs tiles [96, HW];
    per-head logits = one 96-contract matmul per (mb, lh) vs tmpl
    (k-copy rows 0:32 + preloaded U/I masks rows 32:96)
  - logits pre-scaled to base 2 on host (k-weights and kr tables x log2e);
    softmax exp2 split between ScalarE (Exp2 activation) and VectorE
    (custom 2-instruction DVE exp2: exact 2^n int32 carrier + deg-2 poly)
  - A*V per head-pair, 2-way column-tiled (tile_position (0,0)/(0,64)),
    denominators ride as the appended ones-column of vT
  - deferred divisions (fast reciprocal + broadcast), final 1x1 attn conv,
    3x3 conv filling PE gaps at the tail; outputs [conv(256); attn(256)]
"""

import os
import sys

import numpy as np
import ml_dtypes

sys.path.insert(0, "/opt/trn_rl_repo")
sys.path.insert(0, os.path.dirname(os.path.abspath(__file__)))

B, C_IN, H, W = 8, 256, 32, 32
HW = H * W
DK = DV = 256
NH = 8
DKH = DK // NH  # 32
C_OUT = 512
N_CORES = 8
LOG2E = 1.4426950408889634

# which mb-chunks' exp2 go to the vector engine (rest on scalar)
DVE_MBS = (2, 5)

_CACHE = {}

# --- custom DVE exp2 (registered at import) --------------------------------

_DVE_MAGIC = 12582912.0  # 1.5 * 2^23
_DVE_BIAS = 127.0
_DVE_SCALE23 = 8388608.0  # 2^23
_DVE_POLY_A = 0.702941794  # linear coeff
_DVE_POLY_B = 0.239864029  # quadratic coeff


def _register_dve_exp2():
    import concourse.dve_ops as dve_ops_mod
    from concourse.dve_ops import DveOp
    from concourse.dve_spec import C0, C1, C2, One, Spec, Src0, Src1
    from concourse.dve_spec import _has_src1 as has_src1
    from concourse.dve_spec import lower
    from concourse.dve_table_gen import dve_ver_for
    from concourse.dve_uop import DveOpSpec

    if "EXP2_CARRIER_ANT" in dve_ops_mod._SUB_OPCODE_FOR_NAME:
        return tuple(
            next(o for o in dve_ops_mod.OPS if o.name == n)
            for n in ("EXP2_CARRIER_ANT", "EXP2_FINISH_ANT")
        )

    def f32(x):
        return np.asarray(x, np.float32)

    def ref_carrier(in0, in1, s0, s1, imm2):
        t = f32(in0)
        n = f32(f32(t + f32(s0)) - f32(s0))
        return f32(f32(n + f32(s1)) * f32(imm2))

    def ref_finish(in0, in1, s0, s1, imm2):
        t = f32(in0)
        n = f32(f32(t + f32(s0)) - f32(s0))
        f = f32(t - n)
        p = f32(f32(f32(f * f32(s1)) + f32(imm2)) * f + 1.0)
        return f32(p * f32(in1))

    u1 = Src0 + C0
    body_c = ((u1 - C0) + C1) * C2
    u2 = Src0 + C0
    f = Src0 - (u2 - C0)
    body_f = ((f * C1 + C2) * f + One) * Src1

    rows = sorted(dve_ops_mod._SUB_OPCODE_FOR_NAME.values())
    r1, r2 = rows[-1] + 1, rows[-1] + 2
    assert r2 < 0x20
    dve_ops_mod._SUB_OPCODE_FOR_NAME["EXP2_CARRIER_ANT"] = r1
    dve_ops_mod._SUB_OPCODE_FOR_NAME["EXP2_FINISH_ANT"] = r2

    ops = []
    for name, body, ref, row in (
        ("EXP2_CARRIER_ANT", body_c, ref_carrier, r1),
        ("EXP2_FINISH_ANT", body_f, ref_finish, r2),
    ):
        spec = Spec(body=body, reference=ref)
        ver = dve_ver_for("TRN2")
        tmp = DveOpSpec(
            name=name, opcode=row, uops=lower(spec, ver=ver),
            rd1_en=has_src1(spec),
        )
        op = DveOp(name, spec, subdim=False, uops_sha={ver: tmp.sha(ver)})
        dve_ops_mod.OPS.append(op)
        dve_ops_mod.CUSTOM_DVE_SPECS[name] = spec
        ops.append(op)
    return tuple(ops)


def _build():
    import concourse.bass as bass
    import concourse.mybir as mybir
    import concourse.tile as tile
    from concourse import bacc
    from contextlib import ExitStack

    op_carrier, op_finish = _register_dve_exp2()

    f32 = mybir.dt.float32
    bf16 = mybir.dt.bfloat16
    i32 = mybir.dt.int32
    AF = mybir.ActivationFunctionType

    nc = bacc.Bacc("TRN2", target_bir_lowering=False, debug=False,
                   num_devices=N_CORES)

    xp_d = nc.dram_tensor("xp", [2, 128, 34 * 34], bf16, kind="ExternalInput").ap()
    xc_d = nc.dram_tensor("xc", [2, 128, HW], bf16, kind="ExternalInput").ap()
    wqkv_d = nc.dram_tensor("wqkvT", [2, 128, 768], bf16, kind="ExternalInput").ap()
    wout_d = nc.dram_tensor("woutT", [2, 128, 9, 256], bf16, kind="ExternalInput").ap()
    wattn_d = nc.dram_tensor("wattnT", [2, 128, 256], bf16, kind="ExternalInput").ap()
    krh_d = nc.dram_tensor("krh4", [128, 32, 128], bf16, kind="ExternalInput").ap()
    krw_d = nc.dram_tensor("krw4", [128, 32, 128], bf16, kind="ExternalInput").ap()
    masks_d = nc.dram_tensor("masks", [64, 8, 128], bf16, kind="ExternalInput").ap()
    bqkv_d = nc.dram_tensor("bqkv", [128, 4], f32, kind="ExternalInput").ap()
    batt_d = nc.dram_tensor("battn", [128, 2], f32, kind="ExternalInput").ap()
    bout_d = nc.dram_tensor("bout", [128, 2], f32, kind="ExternalInput").ap()
    out_d = nc.dram_tensor("out", [4, 128, HW], f32, kind="ExternalOutput").ap()

    with tile.TileContext(nc) as tc, ExitStack() as ctx:
        wp = ctx.enter_context(tc.tile_pool(name="weights", bufs=1))
        ap_ = ctx.enter_context(tc.tile_pool(name="acts", bufs=1))
        stp = ctx.enter_context(tc.tile_pool(name="st", bufs=4))
        carp = ctx.enter_context(tc.tile_pool(name="carrier", bufs=2))
        dvp = ctx.enter_context(tc.tile_pool(name="div", bufs=2))
        pbig = ctx.enter_context(tc.tile_pool(name="pbig", bufs=3, space="PSUM"))

        # ---- SBUF tiles ----
        xp = wp.tile([128, 2, 34 * 34], bf16)
        xc = wp.tile([128, 2, HW], bf16)
        wqkv = wp.tile([128, 2, 768], bf16)
        wout = wp.tile([128, 2, 9, 256], bf16)
        wattn = wp.tile([128, 2, 256], bf16)
        krh4 = wp.tile([128, 32, 128], bf16)
        krw4 = wp.tile([128, 32, 128], bf16)
        tmpl = wp.tile([96, 4, 8, 128], bf16)  # [k|U|I, slot, mb, m]
        bqkv = wp.tile([128, 4], f32)
        batt = wp.tile([128, 2], f32)
        bout = wp.tile([128, 2], f32)

        qkv = ap_.tile([128, 2, 32, 32], bf16)   # q only, [4-head block, j]
        kblk = ap_.tile([128, 2, HW], bf16)      # k, [4-head block, j]
        rhs = ap_.tile([96, 8, HW], bf16)        # per-head [q; G; W]
        vTe = ap_.tile([128, 8, 8, 33], bf16)    # [m, mb, h, d(+ones)]
        grelB = ap_.tile([128, 2, HW], bf16)
        wrelB = ap_.tile([128, 2, HW], bf16)
        attn = ap_.tile([128, 2, HW], bf16)
        oconv = ap_.tile([128, 2, HW], f32)
        oattn = ap_.tile([128, 2, HW], f32)

        # ---- DMA issue: ring A (sync) critical path, ring B (tensor),
        # ring C (gpsimd) weights, ring D (vector) big late weights ----
        nc.sync.dma_start(xc[:, 0, :], xc_d[0])
        nc.sync.dma_start(wqkv[:, 0, :], wqkv_d[0])
        nc.sync.dma_start(xc[:, 1, :], xc_d[1])
        nc.sync.dma_start(wqkv[:, 1, :], wqkv_d[1])
        nc.sync.dma_start(bqkv[:], bqkv_d[:])
        nc.sync.dma_start(xp[:, 0, :], xp_d[0])
        nc.sync.dma_start(xp[:, 1, :], xp_d[1])
        nc.gpsimd.dma_start(krh4[:], krh_d[:])
        nc.gpsimd.dma_start(krw4[:], krw_d[:])
        for s in range(4):
            nc.gpsimd.dma_start(tmpl[32:96, s, :, :], masks_d[:])
        nc.gpsimd.dma_start(wattn[:, 0, :], wattn_d[0])
        nc.gpsimd.dma_start(wattn[:, 1, :], wattn_d[1])
        nc.gpsimd.dma_start(batt[:], batt_d[:])
        nc.scalar.dma_start(wout[:, 0, :, :], wout_d[0])
        nc.scalar.dma_start(wout[:, 1, :, :], wout_d[1])
        nc.scalar.dma_start(bout[:], bout_d[:])

        nc.gpsimd.memset(vTe[:], 1.0)

        def xv(j):
            return xp[:, j, :].rearrange("p (y x) -> p y x", y=34, x=34)

        def xq(j, half):  # [128, 16, 32] unpadded image half
            return xv(j)[:, 1 + 16 * half:17 + 16 * half, 1:33]

        # ---- qkv: q (ob 0/1), k (ob 2/3); vT after ----
        def qkv_ob(ob):
            # ob 0/1 -> q j=0/1 ; ob 2/3 -> k j=0/1
            ps = pbig.tile([128, HW], f32, tag="big", name=f"qkvps{ob}")
            for half in range(2):
                for j in range(2):
                    nc.tensor.matmul(
                        ps[:, half * 512:(half + 1) * 512],
                        wqkv[:, j, ob * 128:(ob + 1) * 128],
                        xc[:, j, half * 512:(half + 1) * 512],
                        start=(j == 0), stop=(j == 1),
                    )
            jj = ob % 2
            if ob < 2:  # q: big stage into qkv (G/W moving source)
                nc.vector.tensor_scalar_add(
                    qkv[:, jj, :, :].rearrange("p y x -> p (y x)"),
                    ps[:], bqkv[:, ob:ob + 1])
                # per-head copies into rhs rows 0:32 handled later (from qkv)
            else:  # k: big stage into kblk
                nc.scalar.activation(kblk[:, jj, :], ps[:], AF.Identity,
                                     bias=bqkv[:, ob:ob + 1])

        def vT_half(hb):  # m-blocks 4*hb .. 4*hb+3
            pv = pbig.tile([128, HW], f32, tag="big")
            for bb in range(4):
                b = 4 * hb + bb
                for j in range(2):
                    nc.tensor.matmul(
                        pv[:, bb * 256:(bb + 1) * 256],
                        xc[:, j, 128 * b:128 * (b + 1)],
                        wqkv[:, j, 512:768],
                        start=(j == 0), stop=(j == 1),
                    )
            nc.vector.tensor_copy(
                vTe[:, 4 * hb:4 * hb + 4, :, 0:32],
                pv.rearrange("p (b h d) -> p b h d", b=4, h=8, d=32))

        qkv_ob(0)
        qkv_ob(1)
        qkv_ob(2)
        qkv_ob(3)
        vT_half(0)
        vT_half(1)

        # q into rhs rows 0:32 per head (SBUF->SBUF, spread engines)
        for h in range(NH):
            i, j = h % 4, h // 4
            src_q = qkv[32 * i:32 * i + 32, j, :, :].rearrange("p y x -> p (y x)")
            if h % 2:
                nc.scalar.activation(rhs[0:32, h, :], src_q, AF.Copy)
            else:
                nc.vector.tensor_copy(rhs[0:32, h, :], src_q)

        # ---- rel-position G/W via 4-way diagonal tile matmuls ----
        # G[(i,y2), (j,y,x)] = sum_d krh[31+y2-y, d] * q[(i,d), (j,y,x)]
        for yh in range(2):
            pgh = pbig.tile([128, HW], f32, tag="big")
            pgv = pgh.rearrange("p (y c) -> p y c", y=16, c=64)
            for yy in range(16):
                y = 16 * yh + yy
                nc.tensor.matmul(
                    pgv[:, yy, :],
                    krh4[:, y, :],
                    qkv[:, :, y, :],
                    start=True, stop=True,
                )
            nc.scalar.activation(
                grelB[:].rearrange("p j (y x) -> p j y x", y=32, x=32)
                [:, :, 16 * yh:16 * yh + 16, :],
                pgv.rearrange("p y (j x) -> p j y x", j=2, x=32),
                AF.Copy)
        for h in range(NH):
            i, j = h % 4, h // 4
            eng = nc.vector if h % 2 == 0 else nc.scalar
            if eng is nc.scalar:
                nc.scalar.activation(rhs[32:64, h, :],
                                     grelB[32 * i:32 * i + 32, j, :],
                                     AF.Copy)
            else:
                nc.vector.tensor_copy(rhs[32:64, h, :],
                                      grelB[32 * i:32 * i + 32, j, :])
        # W side
        for xh in range(2):
            pw = pbig.tile([128, HW], f32, tag="big")
            pwv = pw.rearrange("p (xx j y) -> p xx j y", xx=16, j=2, y=32)
            for xx in range(16):
                x = 16 * xh + xx
                nc.tensor.matmul(
                    pwv[:, xx, :, :],
                    krw4[:, x, :],
                    qkv[:, :, :, x],
                    start=True, stop=True,
                )
            nc.vector.tensor_copy(
                wrelB[:].rearrange("p j (y x) -> p j y x", y=32, x=32)
                [:, :, :, 16 * xh:16 * xh + 16],
                pwv.rearrange("p xx j y -> p j y xx"),
            )
        for h in range(NH):
            i, j = h % 4, h // 4
            eng = nc.vector if h % 2 == 0 else nc.scalar
            if eng is nc.scalar:
                nc.scalar.activation(rhs[64:96, h, :],
                                     wrelB[32 * i:32 * i + 32, j, :],
                                     AF.Copy)
            else:
                nc.vector.tensor_copy(rhs[64:96, h, :],
                                      wrelB[32 * i:32 * i + 32, j, :])

        # ---- attention: head pairs, logits -> exp2 -> 2-way tiled A*V ----
        pav = ctx.enter_context(tc.tile_pool(name="pav", bufs=1, space="PSUM"))
        pend = []

        def divide(avp, pair):
            a, b = 2 * pair, 2 * pair + 1
            for h, row0 in ((a, 0), (b, 64)):
                i, j = h % 4, h // 4
                hp0 = 32 * i
                dn = dvp.tile([1, HW], f32, tag="dn")
                nc.vector.tensor_copy(dn[:], avp[row0 + 32:row0 + 33, :])
                rdn = dvp.tile([1, HW], f32, tag="rdn")
                nc.vector.reciprocal_approx_fast(rdn[:], dn[:])
                rb = dvp.tile([32, HW], f32, tag="rb")
                nc.gpsimd.partition_broadcast(rb[:], rdn[:])
                nc.vector.tensor_mul(attn[hp0:hp0 + 32, j, :],
                                     avp[row0:row0 + 32, :], rb[:])

        def kcopy(h):
            i, j, s = h % 4, h // 4, h % 4
            nc.vector.tensor_copy(
                tmpl[0:32, s, :, :],
                kblk[32 * i:32 * i + 32, j, :]
                .rearrange("p (m c) -> p m c", m=8, c=128))

        def logits_mb(h, mb, st):
            s = h % 4
            ps = pbig.tile([128, HW], f32, tag="big")
            for lh in range(2):
                nc.tensor.matmul(
                    ps[:, lh * 512:(lh + 1) * 512],
                    tmpl[:, s, mb, :],
                    rhs[:, h, lh * 512:(lh + 1) * 512],
                    start=True, stop=True,
                )
            if mb in DVE_MBS:
                car = carp.tile([128, HW], i32, tag="car")
                nc.vector._custom_dve(
                    op_carrier, out=car[:], in0=ps[:],
                    s0=_DVE_MAGIC, s1=_DVE_BIAS, imm2=_DVE_SCALE23)
                nc.vector._custom_dve(
                    op_finish, out=st[:, mb, :], in0=ps[:],
                    in1=car[:].bitcast(f32),
                    s0=_DVE_MAGIC, s1=_DVE_POLY_B, imm2=_DVE_POLY_A)
            else:
                # 2^t = exp(t * ln2)
                nc.scalar.activation(st[:, mb, :], ps[:], AF.Exp,
                                     scale=0.6931471805599453)

        def av_mb(ctx_p, mb):
            st_a, st_b, avp, pair = ctx_p
            a, b = 2 * pair, 2 * pair + 1
            for lh in range(2):
                nc.tensor.matmul(
                    avp[0:33, lh * 512:(lh + 1) * 512],
                    vTe[:, mb, a, :],
                    st_a[:, mb, lh * 512:(lh + 1) * 512],
                    start=(mb == 0), stop=(mb == 7),
                )
                nc.tensor.matmul(
                    avp[64:97, lh * 512:(lh + 1) * 512],
                    vTe[:, mb, b, :],
                    st_b[:, mb, lh * 512:(lh + 1) * 512],
                    start=(mb == 0), stop=(mb == 7),
                )

        def conv3_ob(ob):
            ps = pbig.tile([128, HW], f32, tag="big", name=f"convps{ob}")
            for half in range(2):
                for j in range(2):
                    for t in range(9):
                        ky, kx = t // 3, t % 3
                        nc.tensor.matmul(
                            ps[:, half * 512:(half + 1) * 512],
                            wout[:, j, t, ob * 128:(ob + 1) * 128],
                            xv(j)[:, half * 16 + ky: half * 16 + ky + 16,
                                  kx: kx + 32],
                            start=((j, t) == (0, 0)), stop=((j, t) == (1, 8)),
                        )
            if ob == 0:
                nc.vector.tensor_scalar_add(oconv[:, ob, :], ps[:],
                                            bout[:, ob:ob + 1])
            else:
                nc.scalar.activation(oconv[:, ob, :], ps[:], AF.Identity,
                                     bias=bout[:, ob:ob + 1])
            for hh in range(2):
                nc.sync.dma_start(out_d[ob, :, hh * 512:(hh + 1) * 512],
                                  oconv[:, ob, hh * 512:(hh + 1) * 512])

        def attnconv_ob(ob):
            ps = pbig.tile([128, HW], f32, tag="big", name=f"attnps{ob}")
            for lh in range(2):
                for j in range(2):
                    nc.tensor.matmul(
                        ps[:, lh * 512:(lh + 1) * 512],
                        wattn[:, j, ob * 128:(ob + 1) * 128],
                        attn[:, j, lh * 512:(lh + 1) * 512],
                        start=(j == 0), stop=(j == 1),
                    )
            nc.vector.tensor_scalar_add(oattn[:, ob, :], ps[:],
                                        batt[:, ob:ob + 1])
            for hh in range(2):
                nc.scalar.dma_start(out_d[2 + ob, :, hh * 512:(hh + 1) * 512],
                                    oattn[:, ob, hh * 512:(hh + 1) * 512])

        kcopy(0)
        kcopy(1)
        prev = None
        for pair in range(4):
            a, b = 2 * pair, 2 * pair + 1
            if pair < 3:
                kcopy(2 * pair + 2)
                kcopy(2 * pair + 3)
            st_a = stp.tile([128, 8, HW], bf16, tag="st")
            st_b = stp.tile([128, 8, HW], bf16, tag="st")
            avp = pav.tile([128, HW], f32, tag="av")
            cur = (st_a, st_b, avp, pair)

            logits_mb(a, 0, st_a)
            logits_mb(a, 1, st_a)
            if prev is not None:
                av_mb(prev, 6)
                av_mb(prev, 7)
                divide(prev[2], prev[3])
            for mb in range(2, 8):
                logits_mb(a, mb, st_a)
            for mb in range(8):
                logits_mb(b, mb, st_b)
                if mb >= 2:
                    av_mb(cur, mb - 2)
            prev = cur

        av_mb(prev, 6)
        conv3_ob(0)
        av_mb(prev, 7)
        divide(prev[2], prev[3])
        attnconv_ob(0)
        conv3_ob(1)
        attnconv_ob(1)

    nc.compile()
    return nc


def _host_inputs(x, w_qkv, b_qkv, w_attn, b_attn, w_out, b_out,
                 key_rel_w, key_rel_h):
    bf = ml_dtypes.bfloat16
    s = DKH ** -0.5
    wq = np.asarray(w_qkv, np.float32)[:, :, 0, 0].copy()   # [768, 256]
    bq = np.asarray(b_qkv, np.float32).copy()
    wq[:DK] *= s
    bq[:DK] *= s
    wq[DK:2 * DK] *= LOG2E          # logits in base 2
    bq[DK:2 * DK] *= LOG2E
    wqkvT = np.ascontiguousarray(wq.T).reshape(2, 128, 768).astype(bf)
    wa = np.asarray(w_attn, np.float32)[:, :, 0, 0]          # [256, 256]
    wattnT = np.ascontiguousarray(wa.T).reshape(2, 128, 256).astype(bf)
    woutT = np.ascontiguousarray(
        np.asarray(w_out, np.float32).transpose(1, 2, 3, 0).reshape(256, 9, 256)
    ).reshape(2, 128, 9, 256).astype(bf)

    # block-diagonal shifted windows: diag[32i+d, y, 32i+y2] = krX*log2e
    def window4(kr):
        krT = np.ascontiguousarray(np.asarray(kr, np.float32).T) * LOG2E
        idx = 31 + np.arange(32)[None, :] - np.arange(32)[:, None]  # [y, y2]
        base = krT[:, idx]                                   # [32d, 32y, 32y2]
        A = np.zeros((4, 32, 32, 4, 32), np.float32)
        for i in range(4):
            A[i, :, :, i, :] = base
        return np.ascontiguousarray(A.reshape(128, 32, 128)).astype(bf)

    krh4 = window4(key_rel_h)
    krw4 = window4(key_rel_w)

    masks = np.zeros((64, 8, 128), np.float32)
    for mb in range(8):
        for jj in range(128):
            masks[mb * 4 + jj // 32, mb, jj] = 1.0  # U rows 0:32 (y2)
    for jj in range(128):
        masks[32 + jj % 32, :, jj] = 1.0            # I rows 32:64 (x2)
    masks = masks.astype(bf)

    bqkv = np.ascontiguousarray(bq[:512].reshape(4, 128).T)           # [128, 4]
    bv = np.asarray(b_qkv, np.float32)[512:768]
    battn = np.asarray(b_attn, np.float32) + wa @ bv       # fold v-bias
    battn = np.ascontiguousarray(battn.reshape(2, 128).T)
    boutm = np.ascontiguousarray(np.asarray(b_out, np.float32).reshape(2, 128).T)

    shared = dict(wqkvT=wqkvT, wattnT=wattnT, woutT=woutT, krh4=krh4,
                  krw4=krw4, masks=masks, bqkv=bqkv, battn=battn,
                  bout=boutm)
    xs = np.asarray(x, np.float32).reshape(B, 2, 128, 32, 32)
    xpad = np.zeros((B, 2, 128, 34, 34), np.float32)
    xpad[:, :, :, 1:33, 1:33] = xs
    xpad = xpad.reshape(B, 2, 128, 34 * 34).astype(bf)
    xcf = xs.reshape(B, 2, 128, HW).astype(bf)
    return [dict(shared, xp=np.ascontiguousarray(xpad[i]),
                 xc=np.ascontiguousarray(xcf[i]))
            for i in range(N_CORES)]


def kernel(**inputs):
    from concourse.bass_utils import run_bass_kernel_spmd
    if "nc" not in _CACHE:
        _CACHE["nc"] = _build()
    nc = _CACHE["nc"]
    in_maps = _host_inputs(**inputs)
    res = run_bass_kernel_spmd(nc, in_maps, list(range(N_CORES)),
                               trace=bool(os.environ.get("BASS_KERNEL_TRACE")))
    _CACHE["last_result"] = res
    outs = [r["out"].reshape(C_OUT, H, W) for r in res.results]
    return np.stack(outs).astype(np.float32)
